# revision 1
# baseline (speedup 1.0000x reference)
"""Trainium2 Bass kernel for banded-cosine-similarity QA span logits.

Contract: kernel(**inputs) takes FULL inputs (sequence_outputs [8,2048,2048] f32,
idxs [8,2] int64) and returns the full output tuple (start_logits, end_logits),
each [8,2048] f32.  Sharding: pure data parallel, one example per NeuronCore.

Per-core computation (S=2048 rows, H=2048 hidden, band W=30):
  dot1 = seq @ q1, dot2 = seq @ q2, nsq = rowsum(seq^2)   (the memory-bound part)
  sim[i,w] = (dot1[i]+dot2[i+w]) / (qnorm*sqrt(nsq[i]+nsq[i+w]))  masked band
  start = rowmax, end = anti-diagonal scatter-max of the row-argmax, plus a
  mean/std sign-flip heuristic.

Engine split for the heavy reductions over the [2048,2048] f32 matrix:
  - ScalarE (ACT): nsq via activation(Square, accum_out)
  - VectorE (DVE): dot1/dot2 via fused tensor_tensor_reduce for some row-tiles
  - TensorE (PE):  remaining row-tiles via on-chip transpose + [128h,2]-column
    matmuls accumulating (dot1,dot2) in PSUM (ACT copies PSUM->SBUF)
"""

import os
import numpy as np
from contextlib import ExitStack

import concourse.bass as bass
import concourse.tile as tile
import concourse.bacc as bacc
from concourse import mybir, masks
from concourse.bass_utils import run_bass_kernel_spmd

f32 = mybir.dt.float32
AF = mybir.ActivationFunctionType
OP = mybir.AluOpType

B = 8
S = 2048
H = 2048
W = 30
P = 128
T = S // P          # 16 row tiles
C = H // P          # 16 h chunks
NEG = -1.0e30

# number of row-tiles whose dots are computed on the PE (transpose) route;
# the rest go through DVE fused multiply-reduce.
N_PE_TILES = int(os.environ.get("KERN_PE_TILES", "0"))
PE_TILES = set(range(0, N_PE_TILES))


KERN_STAGE = int(os.environ.get('KERN_STAGE', '99'))


def _emit(tc, ctx, aps):
    nc = tc.nc
    seq_d = aps["seq"]
    qf_d = aps["qf"]
    qb_d = aps["qb"]
    mask_d = aps["maskadd"]
    rv_d = aps["rv"]
    out_d = aps["out"]
    d2f = aps["d2f"]
    sc_d = aps["sc"]
    scb_d = aps["scb"]
    nsf = aps["nsf"]

    persist = ctx.enter_context(tc.tile_pool(name="persist", bufs=1))
    xpool = ctx.enter_context(tc.tile_pool(name="xpool", bufs=3))
    scr_act_p = ctx.enter_context(tc.tile_pool(name="scr_act", bufs=2))
    scr_dve_p = ctx.enter_context(tc.tile_pool(name="scr_dve", bufs=2))
    sbt_p = ctx.enter_context(tc.tile_pool(name="sbt", bufs=2))
    psT_p = ctx.enter_context(tc.tile_pool(name="psT", bufs=4, space="PSUM"))
    pd_p = ctx.enter_context(tc.tile_pool(name="pd", bufs=2, space="PSUM"))
    pst_p = ctx.enter_context(tc.tile_pool(name="pst", bufs=2, space="PSUM"))
    psh_p = ctx.enter_context(tc.tile_pool(name="psh", bufs=4, space="PSUM"))

    # ---- constants / persistent tiles ----
    ident = persist.tile([P, P], f32)
    masks.make_identity(nc, ident[:])
    # bigI[k, y] = 1 iff y == k + W: slices give shifted identities
    bigI = persist.tile([P, P + 2 * W + P], f32)
    nc.gpsimd.memset(bigI[:], 0.0)
    nc.gpsimd.affine_select(
        out=bigI[:], in_=bigI[:], compare_op=OP.not_equal, fill=1.0,
        base=W, channel_multiplier=1, pattern=[[-1, P + 2 * W + P]])
    ones = persist.tile([P, 1], f32)
    nc.vector.memset(ones[:], 1.0)
    zeros16 = persist.tile([P, T], f32)
    nc.vector.memset(zeros16[:], 0.0)
    negm001 = persist.tile([P, T], f32)
    nc.vector.memset(negm001[:], -0.001)
    ninf_big = persist.tile([P, T * W], f32)
    nc.vector.memset(ninf_big[:], NEG)
    zpad = persist.tile([1, 32], f32)
    nc.vector.memset(zpad[:], 0.0)

    qb_sb = persist.tile([P, 2 * C], f32)
    nc.sync.dma_start(qb_sb[:], qb_d[:])
    mask_sb = persist.tile([P, T * W], f32)
    nc.sync.dma_start(mask_sb[:], mask_d[:])
    rv_sb = persist.tile([P, T], mybir.dt.uint8)
    nc.sync.dma_start(rv_sb[:], rv_d[:])
    # HW DGE mishandles wide 0-step partition broadcasts from DRAM, so
    # replicate across partitions by doubling SBUF->SBUF DMAs instead.
    q12b = persist.tile([P, 2 * H], f32)
    nc.gpsimd.dma_start(q12b[0:1, :], qf_d[:].rearrange("a b -> (a b)").unsqueeze(0))
    k = 1
    while k < P:
        nc.gpsimd.dma_start(q12b[k:2 * k, :], q12b[0:k, :])
        k *= 2
    q1b = q12b[:, 0:H]
    q2b = q12b[:, H:2 * H]

    dot1_cols = persist.tile([P, T], f32)
    dot2_cols = persist.tile([P, T], f32)
    nsq_cols = persist.tile([P, T], f32)

    # ---- qnorm^2 ----
    qscr = persist.tile([P, 2 * C], f32)
    qcol = persist.tile([P, 1], f32)
    nc.scalar.activation(qscr[:], qb_sb[:], AF.Square, accum_out=qcol[:])
    ps_q = pst_p.tile([1, 1], f32, tag="ps_small")
    nc.tensor.matmul(ps_q[:], ones[:], qcol[:], start=True, stop=True)
    qn2_s = persist.tile([1, 1], f32)
    nc.vector.tensor_copy(qn2_s[:], ps_q[:])

    # SBUF partition-broadcast of a [1,1] scalar requires a DRAM bounce
    def bcast_scalar(s11, out_p1, slot):
        nc.sync.dma_start(sc_d[0:1, slot:slot + 1], s11[:])
        nc.sync.dma_start(out_p1[:], sc_d[0:1, slot:slot + 1].broadcast_to([P, 1]))

    qn2_b = persist.tile([P, 1], f32)
    bcast_scalar(qn2_s, qn2_b, 0)

    if KERN_STAGE < 2:
        return
    # ---- phase A: per row-tile reductions ----
    for t in range(T):
        x = xpool.tile([P, H], f32, tag="x")
        eng = nc.sync if t % 2 == 0 else nc.scalar
        eng.dma_start(x[:], seq_d[t * P:(t + 1) * P, :])

        # nsq on ACT
        sa = scr_act_p.tile([P, H], f32, tag="sa")
        nc.scalar.activation(sa[:], x[:], AF.Square,
                             accum_out=nsq_cols[:, t:t + 1])

        if t in PE_TILES:
            # transpose route: PE computes both dots
            sbT = sbt_p.tile([P, H], f32, tag="sbT")
            for g in range(C // 4):
                # 4 chunk transposes share one PSUM bank, one ACT copy out
                pt = psT_p.tile([P, 4 * P], f32, tag="pt")
                for k in range(4):
                    c = g * 4 + k
                    nc.tensor.transpose(pt[:, k * P:(k + 1) * P],
                                        x[:, c * P:(c + 1) * P], ident[:])
                nc.scalar.copy(sbT[:, g * 4 * P:(g + 1) * 4 * P], pt[:])
            pd = pd_p.tile([P, 2], f32, tag="pd")
            for c in range(C):
                nc.tensor.matmul(pd[:], sbT[:, c * P:(c + 1) * P],
                                 qb_sb[:, 2 * c:2 * c + 2],
                                 start=(c == 0), stop=(c == C - 1))
            nc.vector.tensor_copy(dot1_cols[:, t:t + 1], pd[:, 0:1])
            nc.vector.tensor_copy(dot2_cols[:, t:t + 1], pd[:, 1:2])
        else:
            sv = scr_dve_p.tile([P, H], f32, tag="sv")
            nc.vector.scalar_tensor_tensor(
                out=sv[:], in0=x[:], scalar=1.0, in1=q1b,
                op0=OP.mult, op1=OP.mult, accum_out=dot1_cols[:, t:t + 1])
            sv2 = scr_dve_p.tile([P, H], f32, tag="sv")
            nc.vector.scalar_tensor_tensor(
                out=sv2[:], in0=x[:], scalar=1.0, in1=q2b,
                op0=OP.mult, op1=OP.mult, accum_out=dot2_cols[:, t:t + 1])

    if KERN_STAGE < 3:
        return
    # ---- phase B: flatten vectors to DRAM, band-gather back ----
    d2flat_w = bass.AP(d2f.tensor, 0, [[1, P], [P, T]])
    nc.sync.dma_start(d2flat_w, dot2_cols[:])
    nsflat_w = bass.AP(nsf.tensor, 0, [[1, P], [P, T]])
    nc.sync.dma_start(nsflat_w, nsq_cols[:])
    nc.sync.dma_start(bass.AP(d2f.tensor, S, [[32, 1], [1, 32]]), zpad[:])
    nc.sync.dma_start(bass.AP(nsf.tensor, S, [[32, 1], [1, 32]]), zpad[:])

    d2_all = persist.tile([P, T * W], f32)
    nc.sync.dma_start(
        d2_all[:].rearrange("p (t w) -> p t w", w=W),
        bass.AP(d2f.tensor, 0, [[1, P], [P, T], [1, W]]))
    n2_all = persist.tile([P, T * W], f32)
    nc.sync.dma_start(
        n2_all[:].rearrange("p (t w) -> p t w", w=W),
        bass.AP(nsf.tensor, 0, [[1, P], [P, T], [1, W]]))

    if KERN_STAGE < 4:
        return
    # ---- phase C: banded similarity, max, scatter-max ----
    d1v = dot1_cols[:].unsqueeze(2).broadcast_to([P, T, W])
    nsv = nsq_cols[:].unsqueeze(2).broadcast_to([P, T, W])

    s_all = persist.tile([P, T * W], f32)
    nc.vector.tensor_tensor(out=s_all[:].rearrange("p (t w) -> p t w", w=W),
                            in0=n2_all[:].rearrange("p (t w) -> p t w", w=W),
                            in1=nsv, op=OP.add)
    den = persist.tile([P, T * W], f32)
    nc.scalar.activation(den[:], s_all[:], AF.Sqrt, scale=qn2_b[:])
    num = persist.tile([P, T * W], f32)
    nc.vector.tensor_tensor(out=num[:].rearrange("p (t w) -> p t w", w=W),
                            in0=d2_all[:].rearrange("p (t w) -> p t w", w=W),
                            in1=d1v, op=OP.add)
    rden = persist.tile([P, T * W], f32)
    nc.vector.reciprocal(rden[:], den[:])
    simv = persist.tile([P, T * W], f32)
    nc.vector.tensor_tensor(out=simv[:], in0=num[:], in1=rden[:], op=OP.mult)
    simm = persist.tile([P, T * W], f32)
    nc.vector.tensor_tensor(out=simm[:], in0=simv[:], in1=mask_sb[:], op=OP.add)

    smax = persist.tile([P, T], f32)
    nc.vector.tensor_reduce(smax[:], simm[:].rearrange("p (t w) -> p t w", w=W),
                            axis=mybir.AxisListType.X, op=OP.max)

    if KERN_STAGE < 41:
        return
    eq = persist.tile([P, T * W], mybir.dt.uint8)
    nc.vector.tensor_tensor(out=eq[:].rearrange("p (t w) -> p t w", w=W),
                            in0=simm[:].rearrange("p (t w) -> p t w", w=W),
                            in1=smax[:].unsqueeze(2).broadcast_to([P, T, W]),
                            op=OP.is_equal)
    e_all = persist.tile([P, T * W], f32)
    nc.scalar.copy(e_all[:], ninf_big[:])
    nc.vector.copy_predicated(e_all[:], eq[:], simm[:])

    if KERN_STAGE < 42:
        return
    # anti-diagonal scatter-max via PE shifted identities:
    # D_w[p, t] = E[128t + p - w] ; endv = max_w D_w.  Shift-by-w =
    # matmul with bigI slices (exact 0/1 weights; E uses -1e30 not -inf
    # so 0 * E stays 0).  Fake 0s only reach rows e < W < sep0+1, where
    # endv has no real contribution and end_logits is 0 either way.
    e3 = e_all[:].rearrange("p (t w) -> p t w", w=W)
    endv = persist.tile([P, T], f32)
    nc.vector.memset(endv[:], NEG)
    for w in range(W):
        psh = psh_p.tile([P, T], f32, tag="psh")
        nc.tensor.matmul(psh[:], bigI[:, W - w:W - w + P], e3[:, :, w],
                         start=True, stop=(w == 0))
        if w > 0:
            nc.tensor.matmul(psh[:, 1:T], bigI[:, W - w + P:W - w + 2 * P],
                             e3[:, 0:T - 1, w], start=False, stop=True)
        nc.vector.tensor_tensor(out=endv[:], in0=endv[:], in1=psh[:],
                                op=OP.max)

    if KERN_STAGE < 43:
        return
    # end_logits = where(endv == -inf, 0, endv)
    eq2 = persist.tile([P, T], mybir.dt.uint8)
    nc.vector.tensor_tensor(out=eq2[:], in0=endv[:], in1=ninf_big[:, 0:T],
                            op=OP.is_equal)
    end_lg = persist.tile([P, T], f32)
    nc.vector.select(end_lg[:], eq2[:], zeros16[:], endv[:])
    # start_logits = where(row_valid, smax, 0)
    start_lg = persist.tile([P, T], f32)
    nc.vector.select(start_lg[:], rv_sb[:], smax[:], zeros16[:])

    if KERN_STAGE < 6:
        return
    # ---- phase D: stats + flip ----
    stat_row = persist.tile([1, P], f32)

    def cross_max(x16, out11, tagsfx):
        colmax = persist.tile([P, 1], f32, tag="colmax" + tagsfx)
        nc.vector.tensor_reduce(colmax[:], x16[:], axis=mybir.AxisListType.X,
                                op=OP.max)
        nc.sync.dma_start(stat_row[:], colmax[:])
        nc.vector.tensor_reduce(out11[:], stat_row[:],
                                axis=mybir.AxisListType.X, op=OP.max)

    def mean_std(x16, tagsfx):
        colsum = persist.tile([P, 1], f32, tag="cs" + tagsfx)
        nc.vector.tensor_reduce(colsum[:], x16[:], axis=mybir.AxisListType.X,
                                op=OP.add)
        ps = pst_p.tile([1, 1], f32, tag="ps_small")
        nc.tensor.matmul(ps[:], ones[:], colsum[:], start=True, stop=True)
        m = persist.tile([1, 1], f32, tag="m" + tagsfx)
        nc.scalar.mul(m[:], ps[:], 1.0 / S)
        negm = persist.tile([1, 1], f32, tag="nm" + tagsfx)
        nc.scalar.mul(negm[:], m[:], -1.0)
        negm_b = persist.tile([P, 1], f32, tag="nmb" + tagsfx)
        bcast_scalar(negm, negm_b, 1 if tagsfx == "s" else 2)
        scr = persist.tile([P, T], f32, tag="scr" + tagsfx)
        sqcol = persist.tile([P, 1], f32, tag="sq" + tagsfx)
        nc.scalar.activation(scr[:], x16[:], AF.Square, bias=negm_b[:],
                             accum_out=sqcol[:])
        ps2 = pst_p.tile([1, 1], f32, tag="ps_small")
        nc.tensor.matmul(ps2[:], ones[:], sqcol[:], start=True, stop=True)
        var = persist.tile([1, 1], f32, tag="v" + tagsfx)
        nc.scalar.mul(var[:], ps2[:], 1.0 / (S - 1))
        sd = persist.tile([1, 1], f32, tag="sd" + tagsfx)
        nc.scalar.activation(sd[:], var[:], AF.Sqrt)
        thr = persist.tile([1, 1], f32, tag="thr" + tagsfx)
        nc.vector.tensor_tensor(out=thr[:], in0=m[:], in1=sd[:], op=OP.add)
        return thr

    maxs = persist.tile([1, 1], f32)
    cross_max(start_lg, maxs, "s")
    thr_s = mean_std(start_lg, "s")
    thr_e = mean_std(end_lg, "e")
    fl_s = persist.tile([1, 1], mybir.dt.uint8)
    nc.vector.tensor_tensor(out=fl_s[:], in0=maxs[:], in1=thr_s[:], op=OP.is_lt)
    fl_e = persist.tile([1, 1], mybir.dt.uint8)
    nc.vector.tensor_tensor(out=fl_e[:], in0=maxs[:], in1=thr_e[:], op=OP.is_lt)
    flip = persist.tile([1, 1], mybir.dt.uint8)
    nc.vector.tensor_tensor(out=flip[:], in0=fl_s[:], in1=fl_e[:], op=OP.max)
    flip_b = persist.tile([P, 1], mybir.dt.uint8)
    nc.sync.dma_start(scb_d[0:1, 0:1], flip[:])
    nc.sync.dma_start(flip_b[:], scb_d[0:1, 0:1].broadcast_to([P, 1]))

    if KERN_STAGE < 7:
        return
    # ---- phase E: apply flip, write outputs ----
    for k, x16 in enumerate((start_lg, end_lg)):
        negx = persist.tile([P, T], f32, tag=f"negx{k}")
        nc.vector.tensor_scalar_mul(negx[:], x16[:], -1.0)
        isz = persist.tile([P, T], mybir.dt.uint8, tag=f"isz{k}")
        nc.vector.tensor_tensor(out=isz[:], in0=x16[:], in1=zeros16[:],
                                op=OP.is_equal)
        negged = persist.tile([P, T], f32, tag=f"ngd{k}")
        nc.vector.select(negged[:], isz[:], negm001[:], negx[:])
        outv = persist.tile([P, T], f32, tag=f"outv{k}")
        nc.vector.select(outv[:], flip_b[:].broadcast_to([P, T]), negged[:],
                         x16[:])
        nc.sync.dma_start(bass.AP(out_d.tensor, k * S, [[1, P], [P, T]]),
                          outv[:])


_NC_CACHE = {}


def build_program():
    key = (N_PE_TILES, KERN_STAGE)
    if key in _NC_CACHE:
        return _NC_CACHE[key]
    nc = bacc.Bacc("TRN2", target_bir_lowering=False, debug=False)
    aps = {
        "seq": nc.dram_tensor("seq", [S, H], f32, kind="ExternalInput").ap(),
        "qf": nc.dram_tensor("qf", [2, H], f32, kind="ExternalInput").ap(),
        "qb": nc.dram_tensor("qb", [P, 2 * C], f32, kind="ExternalInput").ap(),
        "maskadd": nc.dram_tensor("maskadd", [P, T * W], f32,
                                  kind="ExternalInput").ap(),
        "rv": nc.dram_tensor("rv", [P, T], mybir.dt.uint8,
                             kind="ExternalInput").ap(),
        "out": nc.dram_tensor("out", [2, S], f32, kind="ExternalOutput").ap(),
        "d2f": nc.dram_tensor("d2f", [S + 32], f32).ap(),
        "nsf": nc.dram_tensor("nsf", [S + 32], f32).ap(),
        "sc": nc.dram_tensor("sc", [1, 8], f32).ap(),
        "scb": nc.dram_tensor("scb", [1, 8], mybir.dt.uint8).ap(),
    }
    with tile.TileContext(nc) as tc, ExitStack() as ctx:
        _emit(tc, ctx, aps)
    nc.compile()
    _NC_CACHE[key] = nc
    return nc


def host_prep(seq, idx):
    """Per-core derived inputs from one example. seq [S,H] f32, idx [2] int."""
    sep0, sep1 = int(idx[0]), int(idx[1])
    q1 = np.ascontiguousarray(seq[1])
    q2 = np.ascontiguousarray(seq[sep0 - 1])
    qf = np.stack([q1, q2])                                    # [2,H]
    qb = np.empty((P, 2 * C), np.float32)
    qb[:, 0::2] = q1.reshape(C, P).T
    qb[:, 1::2] = q2.reshape(C, P).T
    i = np.arange(S)[:, None]                                  # [S,1]
    w = np.arange(W)[None, :]
    valid = (i >= sep0 + 1) & (i < sep1) & ((i + w) < sep1)    # [S,W]
    maskadd = np.where(valid, np.float32(0), np.float32(NEG))
    # [S,W] -> [P, T*W] with row r=(128t+p) at [p, t*W+w]
    maskadd = np.ascontiguousarray(
        maskadd.reshape(T, P, W).transpose(1, 0, 2).reshape(P, T * W))
    rv = ((np.arange(S) >= sep0 + 1) & (np.arange(S) < sep1)).astype(np.uint8)
    rv = np.ascontiguousarray(rv.reshape(T, P).T)
    return {"seq": np.ascontiguousarray(seq), "qf": qf, "qb": qb,
            "maskadd": maskadd, "rv": rv}


def kernel(sequence_outputs, idxs):
    sequence_outputs = np.asarray(sequence_outputs, dtype=np.float32)
    idxs = np.asarray(idxs)
    nc = build_program()
    in_maps = [host_prep(sequence_outputs[c], idxs[c]) for c in range(B)]
    res = run_bass_kernel_spmd(nc, in_maps, core_ids=list(range(B)))
    outs = np.stack([res.results[c]["out"] for c in range(B)])  # [B,2,S]
    start = np.ascontiguousarray(outs[:, 0, :])
    end = np.ascontiguousarray(outs[:, 1, :])
    return start, end

